# revision 1
# baseline (speedup 1.0000x reference)
"""Trainium2 Bass kernel for KernelAttention (B=2, N=4096, C=512, H=8) — v2.

Sharding: 8 cores; core j handles batch b=j//4 and head-pair p=j%4
(heads 2p, 2p+1 -> a contiguous 128-column slice of the qkv/head space).
Each core computes q/k/v projections for its heads, full attention over
its batch, and a partial FC projection using only its heads' input
columns. The host sums the 4 partials per batch and adds the bias.

v2 vs baseline:
  - exp split across engines: head0 score tiles -> ACT (exact table
    exp), head1 tiles -> DVE via Schraudolph in the bf16 bit domain:
    bits = rne_i16(A*s + B) written through an int16-bitcast AP and
    read back as bf16 (HW-validated; ~3% elementwise, 6.9e-3 end to
    end vs the 2e-2 gate).
  - software-pipelined emission in 2-unit steps (unit = one head of one
    jb): the DVE exp goes before the step's scores pair and the ACT exp
    after, so both score windows reuse ring slots released by
    previous-step exps — h0/h1 scores become ready together and the
    scheduler keeps their t-interleaved T0/T8 row-tile alternation
    (packed matmul pairs on HW).  pv follows at step m-3.  Scores psum
    rotates through a 3-slot pool tag (6 banks); o accumulators + FC
    psum share a 2-bank tag (8 total).
  - chunk tail: per-head FC packs via a split-partition oT/wfc layout
    (h0 rows 0:64, h1 rows 64:128 -> row-tiled matmul pairs); h0
    normalize on ACT (per-partition AP scale), h1 normalize + head add
    fused in one DVE affine_then_add; evacuations split ACT/DVE.
  - input DMA + projections strip-mined in 4 N-blocks so scores start
    ~25% into the input load.
"""

import numpy as np

B = 2
N = 4096
C = 512
H = 8
DH = 64
SCALE = C ** -0.5
NCORES = 8

ICHUNK = 512            # q rows per chunk
NCHUNK = N // ICHUNK    # 8
NJT = N // 128          # 32 j tiles
JBATCH = 2              # j tiles per unit (psum window [128, 1024])
NJB = NJT // JBATCH     # 16
NU = 2 * NJB            # 32 units per chunk: u = 2*jb + h
NB = 4                  # projection/DMA strip blocks (1024 cols each)

# Schraudolph fast-exp constants (bf16 bit domain; HW int16 convert is
# round-to-nearest): bits = rne(A*s + B) viewed as bf16.
A_DVE = SCALE * 128.0 / float(np.log(2.0))
B_DVE = 127.0 * 128.0 - 366393.0 / 65536.0

# h1 exp goes to DVE when (jb % 16) < DVE_FRAC; else ACT.
DVE_FRAC = 16

_BUILT = None


def _build():
    import concourse.tile as tile
    from concourse import bacc, mybir

    f32 = mybir.dt.float32
    f32r = mybir.dt.float32r
    bf16 = mybir.dt.bfloat16
    i16 = mybir.dt.int16
    EXP = mybir.ActivationFunctionType.Exp
    MULT = mybir.AluOpType.mult
    ADD = mybir.AluOpType.add

    nc = bacc.Bacc("TRN2", target_bir_lowering=False, debug=False,
                   num_devices=NCORES)

    xT = nc.dram_tensor("xT", [C, N], bf16, kind="ExternalInput").ap()
    wq = nc.dram_tensor("wq", [C, 128], bf16, kind="ExternalInput").ap()
    wk = nc.dram_tensor("wk", [C, 128], bf16, kind="ExternalInput").ap()
    wv = nc.dram_tensor("wv", [C, 128], bf16, kind="ExternalInput").ap()
    wfc = nc.dram_tensor("wfc", [128, C], f32r, kind="ExternalInput").ap()
    y = nc.dram_tensor("y", [N, C], f32, kind="ExternalOutput").ap()

    CO = C // 128  # 4 contraction subtiles for the projections

    from contextlib import ExitStack
    with tile.TileContext(nc) as tc, ExitStack() as ctx:
        const = ctx.enter_context(tc.tile_pool(name="const", bufs=1))
        ps_s = ctx.enter_context(tc.tile_pool(name="ps_s", bufs=3, space="PSUM"))
        ps_o = ctx.enter_context(tc.tile_pool(name="ps_o", bufs=2, space="PSUM"))
        pT_pool = ctx.enter_context(tc.tile_pool(name="pT", bufs=4))
        oT_pool = ctx.enter_context(tc.tile_pool(name="oT", bufs=2))
        sm_pool = ctx.enter_context(tc.tile_pool(name="small", bufs=2))
        tmp_pool = ctx.enter_context(tc.tile_pool(name="tmp", bufs=4))
        y_pool = ctx.enter_context(tc.tile_pool(name="ysb", bufs=3))

        rk = [0]

        def rwin():
            rk[0] += 1
            return ps_s.tile([128, 1024], f32, tag="s", name=f"w{rk[0]}")[:]

        # ---- weights to SBUF ----
        wq_sb = const.tile([128, CO, 128], bf16)
        wk_sb = const.tile([128, CO, 128], bf16)
        wv_sb = const.tile([128, CO, 128], bf16)
        wfc_sb = const.tile([128, C], f32r)
        ident = const.tile([128, 128], f32)
        from concourse.masks import make_identity
        make_identity(nc, ident)

        # ---- strip-mined input DMA + projections ----
        xT_sb = const.tile([128, CO, N], bf16)
        qT_sb = const.tile([128, N], bf16)
        kT_sb = const.tile([128, N], bf16)
        # v natural layout + ones columns: vA [j=128, jt=32, 130]
        # cols 0:64 = v_h0, 64 = 1.0, 65:129 = v_h1, 129 = 1.0
        vA_sb = const.tile([128, NJT, 130], bf16)
        nc.vector.memset(vA_sb[:, :, 64:65], 1.0)
        nc.vector.memset(vA_sb[:, :, 129:130], 1.0)

        # issue all input DMAs up front (transfer latency), but emit the
        # nb>=1 projection compute inside the early main-loop steps so the
        # ACT-queue evac copies stop head-of-line-blocking the first exps.
        for nb in range(NB):
            nsl = slice(nb * 1024, (nb + 1) * 1024)
            for co in range(CO):
                nc.sync.dma_start(xT_sb[:, co, nsl],
                                  xT[co * 128:(co + 1) * 128, nsl])
            if nb == 0:
                for w_sb, w_dram in ((wk_sb, wk), (wq_sb, wq), (wv_sb, wv)):
                    for co in range(CO):
                        nc.sync.dma_start(w_sb[:, co, :],
                                          w_dram[co * 128:(co + 1) * 128, :])
                nc.sync.dma_start(wfc_sb[:], wfc[:, :])

        def emit_proj(nb):
            nsl = slice(nb * 1024, (nb + 1) * 1024)
            for dst, w_sb in ((kT_sb, wk_sb), (qT_sb, wq_sb)):
                ps = rwin()
                for half in range(2):
                    isl = slice(nb * 1024 + half * 512,
                                nb * 1024 + (half + 1) * 512)
                    for co in range(CO):
                        nc.tensor.matmul(ps[:, half * 512:(half + 1) * 512],
                                         lhsT=w_sb[:, co, :],
                                         rhs=xT_sb[:, co, isl],
                                         start=(co == 0), stop=(co == CO - 1))
                nc.scalar.copy(dst[:, nsl], ps)
            ps = rwin()
            for s8 in range(8):
                jt = nb * 8 + s8
                for co in range(CO):
                    nc.tensor.matmul(ps[:, s8 * 128:(s8 + 1) * 128],
                                     lhsT=xT_sb[:, co, jt * 128:(jt + 1) * 128],
                                     rhs=wv_sb[:, co, :],
                                     start=(co == 0), stop=(co == CO - 1))
            vsrc = ps.rearrange("p (s8 h d) -> p s8 h d", s8=8, h=2)
            for h in range(2):
                nc.vector.tensor_copy(
                    vA_sb[:, nb * 8:(nb + 1) * 8, h * 65:h * 65 + 64],
                    vsrc[:, :, h, :])

        emit_proj(0)

        # ---- main attention + fc (unit-pipelined emission) ----
        win = {}     # g -> scores window AP
        o_tiles = {}  # ic -> [o_ps_h0, o_ps_h1]

        def emit_scores_pair(g):
            # units g (h0) and g+1 (h1): interleave t so consecutive PE
            # matmuls alternate row-tile T0/T8 and pack on HW.
            ic, u = divmod(g, NU)
            jb = u // 2
            isl = slice(ic * ICHUNK, (ic + 1) * ICHUNK)
            ps = {0: rwin(), 1: rwin()}
            for t in range(JBATCH):
                jt = jb * JBATCH + t
                for h in range(2):
                    hp = slice(h * 64, (h + 1) * 64)
                    nc.tensor.matmul(ps[h][:, t * 512:(t + 1) * 512],
                                     lhsT=kT_sb[hp, jt * 128:(jt + 1) * 128],
                                     rhs=qT_sb[hp, isl],
                                     start=True, stop=True)
            win[g] = ps[0]
            win[g + 1] = ps[1]

        pT_tiles = {}

        def emit_exp(g):
            ic, u = divmod(g, NU)
            jb, h = divmod(u, 2)
            ps = win.pop(g)
            pT = pT_pool.tile([128, 1024], bf16, tag=f"pT{h}", bufs=4,
                              name=f"pT{g}")
            if h == 1 and (jb % 16) < DVE_FRAC:
                nc.vector.tensor_scalar(pT[:].bitcast(i16), ps,
                                        A_DVE, B_DVE, MULT, ADD)
            else:
                nc.scalar.activation(pT[:], ps, EXP, scale=SCALE)
            pT_tiles[g] = pT

        def emit_pv(g):
            ic, u = divmod(g, NU)
            jb, h = divmod(u, 2)
            pT = pT_tiles.pop(g)
            if u == 0:
                o_tiles[ic] = [
                    ps_o.tile([128, 512], f32, tag="o", name=f"o{ic}_{hh}")
                    for hh in range(2)]
            o_ps = o_tiles[ic][h]
            for t in range(JBATCH):
                jt = jb * JBATCH + t
                nc.tensor.matmul(o_ps[:65, :],
                                 lhsT=vA_sb[:, jt, h * 65:(h + 1) * 65],
                                 rhs=pT[:, t * 512:(t + 1) * 512],
                                 start=(jt == 0), stop=(jt == NJT - 1))

        def emit_tail(ic):
            o_ps = o_tiles.pop(ic)
            oT = oT_pool.tile([128, 512], f32r, tag="oT", name=f"oT{ic}")
            # split evacuation across ACT and DVE so the two heads drain
            # in parallel
            nc.scalar.copy(oT[0:64, :], o_ps[0][0:64, :])
            nc.vector.tensor_copy(oT[64:128, :], o_ps[1][0:64, :])
            rs_sb = []
            for h in range(2):
                r = sm_pool.tile([1, 512], f32, tag=f"rs{h}", name=f"rs{ic}_{h}")
                (nc.vector.tensor_copy if h == 0 else nc.scalar.copy)(
                    r[:], o_ps[h][64:65, :])
                rs_sb.append(r)
            rsP = ps_o.tile([128, 8], f32, tag="o", name=f"rsP{ic}")
            for sub in range(4):
                for h in range(2):
                    nc.tensor.transpose(rsP[:, sub * 2 + h:sub * 2 + h + 1],
                                        rs_sb[h][:, sub * 128:(sub + 1) * 128],
                                        ident[0:1, 0:1])
            rs_f = sm_pool.tile([128, 8], f32, tag="rsf", name=f"rsf{ic}")
            nc.vector.tensor_copy(rs_f[:], rsP[:])
            rcp = sm_pool.tile([128, 8], f32, tag="rcp", name=f"rcp{ic}")
            nc.vector.reciprocal(rcp[:], rs_f[:])

            for sub in range(4):
                y_ps = []
                for h in range(2):
                    yp = ps_o.tile([128, 512], f32, tag="o",
                                   name=f"y{ic}_{sub}_{h}")
                    hp = slice(h * 64, (h + 1) * 64)
                    nc.tensor.matmul(yp[:],
                                     lhsT=oT[hp, sub * 128:(sub + 1) * 128],
                                     rhs=wfc_sb[hp, :],
                                     start=True, stop=True)
                    y_ps.append(yp)
                t1 = tmp_pool.tile([128, 512], f32, tag="t1",
                                   name=f"t1_{ic}_{sub}")
                nc.scalar.mul(t1[:], y_ps[0][:], rcp[:, sub * 2:sub * 2 + 1])
                ysb = y_pool.tile([128, 512], f32, tag="ysb",
                                  name=f"ysb{ic}_{sub}")
                nc.vector.affine_then_add(ysb[:], y_ps[1][:], t1[:],
                                          rcp[:, sub * 2 + 1:sub * 2 + 2], 0.0)
                r0 = ic * ICHUNK + sub * 128
                nc.sync.dma_start(y[r0:r0 + 128, :], ysb[:])

        NG = NCHUNK * NU  # 256 units
        NS = NG // 2      # 128 steps of 2 units
        # exp emission straddles the scores: the odd-unit (DVE) exp goes
        # before, the even-unit (ACT) exp after.  Both score windows of a
        # step then reuse slots released by PREVIOUS-step exps, so h0/h1
        # scores become ready together and keep their packed T0/T8
        # alternation in the scheduled order.
        for m in range(NS + 4):
            if 0 <= 2 * m - 1 < NG:
                emit_exp(2 * m - 1)
            # inject remaining projection blocks after this step's first
            # exp: all three reused ring slots then have their reader exps
            # already emitted (units 2m-3, 2m-2, 2m-1).
            if m in (3, 7, 11):
                emit_proj(m // 4 + 1)
            if m < NS:
                emit_scores_pair(2 * m)
            if 0 <= 2 * m < NG:
                emit_exp(2 * m)
            for k in (2 * m - 6, 2 * m - 5):
                if 0 <= k < NG:
                    emit_pv(k)
                    # tail(ic) right after the last pv of chunk ic, before
                    # pv(ic+1, 0) allocates the next o tiles.
                    if k % NU == NU - 1:
                        emit_tail(k // NU)

    nc.compile()
    return nc


def _get_built():
    global _BUILT
    if _BUILT is None:
        _BUILT = _build()
    return _BUILT


def _in_maps(x, w_qkv, w_fc):
    import ml_dtypes
    bf = ml_dtypes.bfloat16
    in_maps = []
    for j in range(NCORES):
        b = j // 4
        p = j % 4
        cs = slice(p * 128, (p + 1) * 128)
        in_maps.append({
            "xT": np.ascontiguousarray(x[b].T).astype(bf),
            "wq": np.ascontiguousarray(w_qkv[cs, :].T).astype(bf),
            "wk": np.ascontiguousarray(w_qkv[C + p * 128:C + (p + 1) * 128, :].T).astype(bf),
            "wv": np.ascontiguousarray(w_qkv[2 * C + p * 128:2 * C + (p + 1) * 128, :].T).astype(bf),
            "wfc": np.ascontiguousarray(w_fc[:, cs].T),
        })
    return in_maps


def kernel(x, w_qkv, w_fc, b_fc):
    from concourse import bass_utils

    nc = _get_built()
    res = bass_utils.run_bass_kernel_spmd(nc, _in_maps(x, w_qkv, w_fc),
                                          core_ids=list(range(NCORES)))
    y = np.zeros((B, N, C), dtype=np.float32)
    for j in range(NCORES):
        y[j // 4] += res.results[j]["y"]
    y += b_fc.astype(np.float32)
    return y



# revision 3
# speedup vs baseline: 1.1062x; 1.1062x over previous
"""Trainium2 Bass kernel for KernelAttention (B=2, N=4096, C=512, H=8) — v3.

Sharding: 8 cores; core j handles batch b=j//4 and head-pair p=j%4
(heads 2p, 2p+1 -> a contiguous 128-column slice of the qkv/head space).
Each core computes q/k/v projections for its heads, full attention over
its batch, and a partial FC projection using only its heads' input
columns. The host sums the 4 partials per batch and adds the bias.

v3 vs v2 (trace-driven, see HW profile):
  - HW model: every matmul costs ~N_out_cols cycles (stream/drain
    bound); K/M packing does not overlap in this regime.  So: minimize
    column-streams, and kill the slow ones.
  - fc: f32r matmuls measured 500-790ns each (vs 215 for bf16) -> oT
    and wfc now bf16.  oT is stored per head as [65, 512] including the
    denominator row 64; wfc per head is [65, 512] with a zero row 64 so
    the denominator doesn't pollute y.
  - denominators: the old path (PE transposes of [1,512] rows, 64 tiny
    matmuls + psum tile + extra copies) is replaced by one N=1 matmul
    per (sub, h) with the SAME fc lhsT and a one-hot rhs column ec
    (row 64) -> rsP[128 i, 8].  One DVE reciprocal gives all 8 scale
    columns.
  - fc psum: y windows [128, 1024] (y_h0 | y_h1 in two banks) rotate
    through the scores ring tag instead of fighting the o-accumulator
    tag; the tail no longer serializes on 2 psum banks, which was
    idling the PE at every chunk boundary (HAM re-throttle to K=4/8,
    ~10us of half-clock per chunk).
  - exp split rebalanced: ACT is faster per tile (1017ns vs 1223ns
    measured), so ACT now takes 9/16 of h1 tiles too (DVE_FRAC=14).
"""

import numpy as np

B = 2
N = 4096
C = 512
H = 8
DH = 64
SCALE = C ** -0.5
NCORES = 8

ICHUNK = 512            # q rows per chunk
NCHUNK = N // ICHUNK    # 8
NJT = N // 128          # 32 j tiles
JBATCH = 2              # j tiles per unit (psum window [128, 1024])
NJB = NJT // JBATCH     # 16
NU = 2 * NJB            # 32 units per chunk: u = 2*jb + h
NB = 4                  # projection/DMA strip blocks (1024 cols each)

# Schraudolph fast-exp constants (bf16 bit domain; HW int16 convert is
# round-to-nearest): bits = rne(A*s + B) viewed as bf16.
A_DVE = SCALE * 128.0 / float(np.log(2.0))
B_DVE = 127.0 * 128.0 - 366393.0 / 65536.0

# h1 exp goes to DVE when (jb % 16) < DVE_FRAC; else ACT.
DVE_FRAC = 14

_BUILT = None


def _build():
    import concourse.tile as tile
    from concourse import bacc, mybir

    f32 = mybir.dt.float32
    bf16 = mybir.dt.bfloat16
    i16 = mybir.dt.int16
    EXP = mybir.ActivationFunctionType.Exp
    MULT = mybir.AluOpType.mult
    ADD = mybir.AluOpType.add

    nc = bacc.Bacc("TRN2", target_bir_lowering=False, debug=False,
                   num_devices=NCORES)

    xT = nc.dram_tensor("xT", [C, N], bf16, kind="ExternalInput").ap()
    wq = nc.dram_tensor("wq", [C, 128], bf16, kind="ExternalInput").ap()
    wk = nc.dram_tensor("wk", [C, 128], bf16, kind="ExternalInput").ap()
    wv = nc.dram_tensor("wv", [C, 128], bf16, kind="ExternalInput").ap()
    wfc0 = nc.dram_tensor("wfc0", [65, C], bf16, kind="ExternalInput").ap()
    wfc1 = nc.dram_tensor("wfc1", [65, C], bf16, kind="ExternalInput").ap()
    y = nc.dram_tensor("y", [N, C], f32, kind="ExternalOutput").ap()

    CO = C // 128  # 4 contraction subtiles for the projections

    from contextlib import ExitStack
    with tile.TileContext(nc) as tc, ExitStack() as ctx:
        const = ctx.enter_context(tc.tile_pool(name="const", bufs=1))
        ps_s = ctx.enter_context(tc.tile_pool(name="ps_s", bufs=3, space="PSUM"))
        ps_o = ctx.enter_context(tc.tile_pool(name="ps_o", bufs=2, space="PSUM"))
        pT_pool = ctx.enter_context(tc.tile_pool(name="pT", bufs=4))
        oT_pool = ctx.enter_context(tc.tile_pool(name="oT", bufs=2))
        sm_pool = ctx.enter_context(tc.tile_pool(name="small", bufs=2))
        tmp_pool = ctx.enter_context(tc.tile_pool(name="tmp", bufs=4))
        y_pool = ctx.enter_context(tc.tile_pool(name="ysb", bufs=3))

        rk = [0]

        def rwin():
            rk[0] += 1
            return ps_s.tile([128, 1024], f32, tag="s", name=f"w{rk[0]}")[:]

        # ---- weights to SBUF ----
        wq_sb = const.tile([128, CO, 128], bf16)
        wk_sb = const.tile([128, CO, 128], bf16)
        wv_sb = const.tile([128, CO, 128], bf16)
        wfc_sb = [const.tile([65, C], bf16, name=f"wfc_sb{h}")
                  for h in range(2)]
        # one-hot column selecting the denominator row of oT
        ec = const.tile([65, 1], bf16)
        nc.vector.memset(ec[:], 0.0)
        nc.vector.memset(ec[64:65, :], 1.0)

        # ---- strip-mined input DMA + projections ----
        xT_sb = const.tile([128, CO, N], bf16)
        qT_sb = const.tile([128, N], bf16)
        kT_sb = const.tile([128, N], bf16)
        # v natural layout + ones columns: vA [j=128, jt=32, 130]
        # cols 0:64 = v_h0, 64 = 1.0, 65:129 = v_h1, 129 = 1.0
        vA_sb = const.tile([128, NJT, 130], bf16)
        nc.vector.memset(vA_sb[:, :, 64:65], 1.0)
        nc.vector.memset(vA_sb[:, :, 129:130], 1.0)

        # issue all input DMAs up front (transfer latency), but emit the
        # nb>=1 projection compute inside the early main-loop steps so the
        # ACT-queue evac copies stop head-of-line-blocking the first exps.
        for nb in range(NB):
            nsl = slice(nb * 1024, (nb + 1) * 1024)
            for co in range(CO):
                nc.sync.dma_start(xT_sb[:, co, nsl],
                                  xT[co * 128:(co + 1) * 128, nsl])
            if nb == 0:
                for w_sb, w_dram in ((wk_sb, wk), (wq_sb, wq), (wv_sb, wv)):
                    for co in range(CO):
                        nc.sync.dma_start(w_sb[:, co, :],
                                          w_dram[co * 128:(co + 1) * 128, :])
                nc.sync.dma_start(wfc_sb[0][:], wfc0[:, :])
                nc.sync.dma_start(wfc_sb[1][:], wfc1[:, :])

        def emit_proj(nb):
            nsl = slice(nb * 1024, (nb + 1) * 1024)
            for dst, w_sb in ((kT_sb, wk_sb), (qT_sb, wq_sb)):
                ps = rwin()
                for half in range(2):
                    isl = slice(nb * 1024 + half * 512,
                                nb * 1024 + (half + 1) * 512)
                    for co in range(CO):
                        nc.tensor.matmul(ps[:, half * 512:(half + 1) * 512],
                                         lhsT=w_sb[:, co, :],
                                         rhs=xT_sb[:, co, isl],
                                         start=(co == 0), stop=(co == CO - 1))
                nc.scalar.copy(dst[:, nsl], ps)
            ps = rwin()
            for s8 in range(8):
                jt = nb * 8 + s8
                for co in range(CO):
                    nc.tensor.matmul(ps[:, s8 * 128:(s8 + 1) * 128],
                                     lhsT=xT_sb[:, co, jt * 128:(jt + 1) * 128],
                                     rhs=wv_sb[:, co, :],
                                     start=(co == 0), stop=(co == CO - 1))
            vsrc = ps.rearrange("p (s8 h d) -> p s8 h d", s8=8, h=2)
            for h in range(2):
                nc.vector.tensor_copy(
                    vA_sb[:, nb * 8:(nb + 1) * 8, h * 65:h * 65 + 64],
                    vsrc[:, :, h, :])

        emit_proj(0)

        # ---- main attention + fc (unit-pipelined emission) ----
        win = {}     # g -> scores window AP
        o_tiles = {}  # ic -> [o_ps_h0, o_ps_h1]

        def emit_scores_pair(g):
            # units g (h0) and g+1 (h1): interleave t so consecutive PE
            # matmuls alternate row-tile T0/T8 and pack on HW.
            ic, u = divmod(g, NU)
            jb = u // 2
            isl = slice(ic * ICHUNK, (ic + 1) * ICHUNK)
            ps = {0: rwin(), 1: rwin()}
            for t in range(JBATCH):
                jt = jb * JBATCH + t
                for h in range(2):
                    hp = slice(h * 64, (h + 1) * 64)
                    nc.tensor.matmul(ps[h][:, t * 512:(t + 1) * 512],
                                     lhsT=kT_sb[hp, jt * 128:(jt + 1) * 128],
                                     rhs=qT_sb[hp, isl],
                                     start=True, stop=True)
            win[g] = ps[0]
            win[g + 1] = ps[1]

        pT_tiles = {}

        def emit_exp(g):
            ic, u = divmod(g, NU)
            jb, h = divmod(u, 2)
            ps = win.pop(g)
            pT = pT_pool.tile([128, 1024], bf16, tag=f"pT{h}", bufs=4,
                              name=f"pT{g}")
            if h == 1 and (jb % 16) < DVE_FRAC:
                nc.vector.tensor_scalar(pT[:].bitcast(i16), ps,
                                        A_DVE, B_DVE, MULT, ADD)
            else:
                nc.scalar.activation(pT[:], ps, EXP, scale=SCALE)
            pT_tiles[g] = pT

        def emit_pv(g):
            ic, u = divmod(g, NU)
            jb, h = divmod(u, 2)
            pT = pT_tiles.pop(g)
            if u == 0:
                o_tiles[ic] = [
                    ps_o.tile([128, 512], f32, tag="o", name=f"o{ic}_{hh}")
                    for hh in range(2)]
            o_ps = o_tiles[ic][h]
            for t in range(JBATCH):
                jt = jb * JBATCH + t
                nc.tensor.matmul(o_ps[:65, :],
                                 lhsT=vA_sb[:, jt, h * 65:(h + 1) * 65],
                                 rhs=pT[:, t * 512:(t + 1) * 512],
                                 start=(jt == 0), stop=(jt == NJT - 1))

        def emit_tail(ic):
            o_ps = o_tiles.pop(ic)
            oT0 = oT_pool.tile([65, 512], bf16, tag="oT0", bufs=2,
                               name=f"oT0_{ic}")
            oT1 = oT_pool.tile([65, 512], bf16, tag="oT1", bufs=2,
                               name=f"oT1_{ic}")
            # split evacuation across ACT and DVE so the two heads drain
            # in parallel; row 64 carries the softmax denominators.
            nc.scalar.copy(oT0[:], o_ps[0][0:65, :])
            nc.vector.tensor_copy(oT1[:], o_ps[1][0:65, :])
            oTs = (oT0, oT1)
            # denominators -> [i-partition] layout via N=1 matmuls with a
            # one-hot rhs (row 64), reusing the fc lhsT slices.
            rsP = ps_o.tile([128, 8], f32, tag="o", name=f"rsP{ic}")
            for sub in range(4):
                for h in range(2):
                    nc.tensor.matmul(rsP[:, sub * 2 + h:sub * 2 + h + 1],
                                     lhsT=oTs[h][:, sub * 128:(sub + 1) * 128],
                                     rhs=ec[:], start=True, stop=True)
            rcp = sm_pool.tile([128, 8], f32, tag="rcp", name=f"rcp{ic}")
            nc.vector.reciprocal(rcp[:], rsP[:])

            for sub in range(4):
                yw = rwin()
                for h in range(2):
                    nc.tensor.matmul(yw[:, h * 512:(h + 1) * 512],
                                     lhsT=oTs[h][:, sub * 128:(sub + 1) * 128],
                                     rhs=wfc_sb[h][:], start=True, stop=True)
                t1 = tmp_pool.tile([128, 512], f32, tag="t1",
                                   name=f"t1_{ic}_{sub}")
                nc.scalar.mul(t1[:], yw[:, 0:512], rcp[:, sub * 2:sub * 2 + 1])
                ysb = y_pool.tile([128, 512], f32, tag="ysb",
                                  name=f"ysb{ic}_{sub}")
                nc.vector.affine_then_add(ysb[:], yw[:, 512:1024], t1[:],
                                          rcp[:, sub * 2 + 1:sub * 2 + 2], 0.0)
                r0 = ic * ICHUNK + sub * 128
                nc.sync.dma_start(y[r0:r0 + 128, :], ysb[:])

        NG = NCHUNK * NU  # 256 units
        NS = NG // 2      # 128 steps of 2 units
        # exp emission straddles the scores: the odd-unit (DVE) exp goes
        # before, the even-unit (ACT) exp after.  Both score windows of a
        # step then reuse slots released by PREVIOUS-step exps, so h0/h1
        # scores become ready together and keep their packed T0/T8
        # alternation in the scheduled order.
        for m in range(NS + 4):
            if 0 <= 2 * m - 1 < NG:
                emit_exp(2 * m - 1)
            # inject remaining projection blocks after this step's first
            # exp: all three reused ring slots then have their reader exps
            # already emitted (units 2m-3, 2m-2, 2m-1).
            if m in (3, 7, 11):
                emit_proj(m // 4 + 1)
            if m < NS:
                emit_scores_pair(2 * m)
            if 0 <= 2 * m < NG:
                emit_exp(2 * m)
            for k in (2 * m - 6, 2 * m - 5):
                if 0 <= k < NG:
                    emit_pv(k)
                    # tail(ic) right after the last pv of chunk ic, before
                    # pv(ic+1, 0) allocates the next o tiles.
                    if k % NU == NU - 1:
                        emit_tail(k // NU)

    nc.compile()
    return nc


def _get_built():
    global _BUILT
    if _BUILT is None:
        _BUILT = _build()
    return _BUILT


def _in_maps(x, w_qkv, w_fc):
    import ml_dtypes
    bf = ml_dtypes.bfloat16
    in_maps = []
    for j in range(NCORES):
        b = j // 4
        p = j % 4
        cs = slice(p * 128, (p + 1) * 128)
        wfcT = np.ascontiguousarray(w_fc[:, cs].T)  # [128 k, 512 c]
        z = np.zeros((1, C), dtype=np.float32)
        in_maps.append({
            "xT": np.ascontiguousarray(x[b].T).astype(bf),
            "wq": np.ascontiguousarray(w_qkv[cs, :].T).astype(bf),
            "wk": np.ascontiguousarray(w_qkv[C + p * 128:C + (p + 1) * 128, :].T).astype(bf),
            "wv": np.ascontiguousarray(w_qkv[2 * C + p * 128:2 * C + (p + 1) * 128, :].T).astype(bf),
            "wfc0": np.vstack([wfcT[0:64], z]).astype(bf),
            "wfc1": np.vstack([wfcT[64:128], z]).astype(bf),
        })
    return in_maps


def kernel(x, w_qkv, w_fc, b_fc):
    from concourse import bass_utils

    nc = _get_built()
    res = bass_utils.run_bass_kernel_spmd(nc, _in_maps(x, w_qkv, w_fc),
                                          core_ids=list(range(NCORES)))
    y = np.zeros((B, N, C), dtype=np.float32)
    for j in range(NCORES):
        y[j // 4] += res.results[j]["y"]
    y += b_fc.astype(np.float32)
    return y
